# revision 1
# baseline (speedup 1.0000x reference)
import numpy as np

# nn_EAS4_46986942218814 — focused linear attention + MixFFN block.
# Shapes are fixed by the problem: x (8, 512, 64, 64) f32.
B, C, H, W = 8, 512, 64, 64
N = H * W
HID = 4 * C
FOCUS = 3.0


def _erf(x):
    try:
        from scipy.special import erf
        return erf(x).astype(np.float32)
    except Exception:
        # Abramowitz–Stegun 7.1.26 fallback (abs err < 1.5e-7)
        sign = np.sign(x)
        ax = np.abs(x)
        t = 1.0 / (1.0 + 0.3275911 * ax)
        y = 1.0 - (((((1.061405429 * t - 1.453152027) * t) + 1.421413741) * t
                    - 0.284496736) * t + 0.254829592) * t * np.exp(-ax * ax)
        return (sign * y).astype(np.float32)


def _ln(x, g, b, eps=1e-5):
    m = x.mean(-1, keepdims=True, dtype=np.float32)
    v = ((x - m) ** 2).mean(-1, keepdims=True, dtype=np.float32)
    return (x - m) / np.sqrt(v + eps) * g + b


def _softmax(x, axis):
    x = x - x.max(axis=axis, keepdims=True)
    e = np.exp(x)
    return e / e.sum(axis=axis, keepdims=True)


def kernel(x, Wq, bq, Wk, bk, Wv, bv, scale, fc1_w, fc1_b, dw_w, dw_b,
           fc2_w, fc2_b, ln1_g, ln1_b, lnm_g, lnm_b):
    x = np.asarray(x, np.float32)
    b = x.shape[0]
    xf = x.reshape(b, C, N)                      # (B, C, N)

    # 1x1 convs as matmuls over the channel dim, then RAW reshape to (B, N, C)
    def conv1(Wm, bm):
        out = np.einsum('oc,bcn->bon', np.asarray(Wm, np.float32), xf,
                        optimize=True) + np.asarray(bm, np.float32)[None, :, None]
        return out.reshape(b, N, C)              # raw reshape, no permute

    q = _softmax(conv1(Wq, bq), axis=1)
    k = _softmax(conv1(Wk, bk), axis=2)
    v = conv1(Wv, bv)

    q = np.maximum(q, 0.0) + 1e-6
    k = np.maximum(k, 0.0) + 1e-6
    sc = np.log1p(np.exp(np.asarray(scale, np.float32)))   # softplus
    q = q / sc
    k = k / sc

    q_norm = np.linalg.norm(q, axis=-1, keepdims=True)
    k_norm = np.linalg.norm(k, axis=-1, keepdims=True)
    q = q ** FOCUS
    k = k ** FOCUS
    q = q / np.linalg.norm(q, axis=-1, keepdims=True) * q_norm
    k = k / np.linalg.norm(k, axis=-1, keepdims=True) * k_norm

    z = 1.0 / (np.einsum('bic,bc->bi', q, k.sum(axis=1), optimize=True) + 1e-6)
    kv = np.einsum('bjc,bjd->bcd', k, v, optimize=True)
    attn = np.einsum('bic,bcd->bid', q, kv, optimize=True) * z[:, :, None]

    shortcut = np.transpose(x, (0, 2, 3, 1)).reshape(b, N, C)
    enhanced = shortcut + attn

    # MixFFN
    t = _ln(enhanced, np.asarray(lnm_g, np.float32), np.asarray(lnm_b, np.float32))
    a = t @ np.asarray(fc1_w, np.float32) + np.asarray(fc1_b, np.float32)  # (B,N,HID)

    # depthwise 3x3 SAME conv on (B, HID, H, W)
    ai = np.transpose(a, (0, 2, 1)).reshape(b, HID, H, W)
    pad = np.zeros((b, HID, H + 2, W + 2), np.float32)
    pad[:, :, 1:-1, 1:-1] = ai
    wdw = np.asarray(dw_w, np.float32)           # (HID, 1, 3, 3)
    dw = np.zeros((b, HID, H, W), np.float32)
    for dy in range(3):
        for dx in range(3):
            dw += pad[:, :, dy:dy + H, dx:dx + W] * wdw[None, :, 0, dy, dx, None, None]
    dw += np.asarray(dw_b, np.float32)[None, :, None, None]
    dw = np.transpose(dw.reshape(b, HID, N), (0, 2, 1))   # (B, N, HID)

    h1 = _ln(dw + a, np.asarray(ln1_g, np.float32), np.asarray(ln1_b, np.float32))
    ax = 0.5 * h1 * (1.0 + _erf(h1 / np.sqrt(np.float32(2.0))))  # exact gelu
    mlp_out = ax @ np.asarray(fc2_w, np.float32) + np.asarray(fc2_b, np.float32)
    out = enhanced + mlp_out
    return out.reshape(b, H, W, C).astype(np.float32)



# revision 20
# speedup vs baseline: 4.4633x; 4.4633x over previous
"""Trainium2 Bass kernel for nn_EAS4_46986942218814.

Focused linear attention + MixFFN block, B=8 batches data-parallel over 8
NeuronCores (one batch per core).

Key layout trick: the reference's *raw* reshape of the 1x1-conv output
(C,N)->(N,C) means  q(8a+b', c) = QC[a, 512b'+c].  So:
  - K/V tiles in (n-part, c-free) layout are exactly the conv-PSUM blocks
    KC[a-sub, 512b':...] (rows n = 8a+b') -- no transpose.
  - Q is computed directly in a permuted-n "(c-part, j-free)" layout
    (j = 512b'+a) via  out = Xb'^T @ WqT  -- no transpose; softmax-over-n
    becomes a free-dim reduction, all n-order-agnostic attention math stays
    in pi-order, and the un-permute is folded into the shortcut-add AP.
  - FFN runs in (hid-part, n-free) layout: fc1/fc2 contract over the
    partition dim, the depthwise 3x3 is 9 fused scalar_tensor_tensor MACs
    over a zero-gap padded (66-wide rows) buffer, LN stats over the
    partition dim are ones-matmuls.
Matmuls run in bf16 (1 cycle/row).  LN gamma/beta of the MLP norm are
folded into fc1 weights on the host.
"""

import sys

for _p in ("/opt/trn_rl_repo", "/root/.axon_site/_ro/trn_rl_repo"):
    if _p not in sys.path:
        sys.path.insert(0, _p)

from contextlib import ExitStack

import ml_dtypes
import numpy as np

import concourse.bass as bass
import concourse.bacc as bacc
import concourse.mybir as mybir
import concourse.tile as tile

B, C, H, W = 8, 512, 64, 64
N = H * W          # 4096
HID = 4 * C        # 2048
NCT = C // 128     # 4   c-partition tiles
NCH = HID // 128   # 16  hid-partition tiles
PW = W + 2         # 66  padded row width
F32 = mybir.dt.float32
BF16 = mybir.dt.bfloat16
BFNP = ml_dtypes.bfloat16
AX = mybir.AxisListType
OP = mybir.AluOpType
AF = mybir.ActivationFunctionType


def emit(tc, io, bq_nonzero=False, dbg=False):
    nc = tc.nc
    with ExitStack() as ctx:
        _emit(tc, nc, ctx, io, bq_nonzero, dbg)


def _bcast(nc, ones_row, psum_ap, row_ap):
    """Broadcast a bf16 SBUF row (1, n) to PSUM (128, n) via K=1 matmul."""
    nc.tensor.matmul(psum_ap, ones_row[:], row_ap, start=True, stop=True,
                     skip_group_check=True)


def _rt(rowp, i, dtype=F32):
    return rowp.tile([1, 512], dtype, tag=f"rt{i}", name=f"rt{i}")


def _row_norm_stats(nc, rowp, Sm, Sm2, inv_n, ln_eps, ln):
    """From PSUM sums (1,ln): bf16 rows (r, m*r) for LN normalize."""
    m_row = _rt(rowp, 0)
    nc.vector.tensor_scalar(m_row[:, :ln], Sm, inv_n, None, op0=OP.mult)
    s2_row = _rt(rowp, 1)
    nc.vector.tensor_scalar(s2_row[:, :ln], Sm2, inv_n, None, op0=OP.mult)
    m2_row = _rt(rowp, 2)
    nc.vector.tensor_tensor(m2_row[:, :ln], m_row[:, :ln], m_row[:, :ln],
                            op=OP.mult)
    var_row = _rt(rowp, 3)
    nc.vector.tensor_tensor(var_row[:, :ln], s2_row[:, :ln], m2_row[:, :ln],
                            op=OP.subtract)
    sd_row = _rt(rowp, 1)
    nc.scalar.activation(sd_row[:, :ln], var_row[:, :ln], AF.Sqrt, bias=ln_eps[:])
    r_row = _rt(rowp, 2)
    nc.vector.reciprocal(r_row[:, :ln], sd_row[:, :ln])
    r_rowb = rowp.tile([1, 512], BF16, tag="rtb0", name="rtb0")
    nc.vector.tensor_copy(r_rowb[:, :ln], r_row[:, :ln])
    m_rowb = rowp.tile([1, 512], BF16, tag="rtb1", name="rtb1")
    nc.vector.tensor_copy(m_rowb[:, :ln], m_row[:, :ln])
    return r_rowb, m_rowb


def _emit(tc, nc, ctx, io, bq_nonzero, dbg=False):
    consts = ctx.enter_context(tc.tile_pool(name="consts", bufs=1))
    small = ctx.enter_context(tc.tile_pool(name="small", bufs=4))
    rowp = ctx.enter_context(tc.tile_pool(name="rowp", bufs=1))

    ones_col = consts.tile([128, 1], BF16, tag="ones_col", name="ones_col")
    nc.vector.memset(ones_col[:], 1.0)
    ones_row = consts.tile([1, 128], BF16, tag="ones_row", name="ones_row")
    nc.vector.memset(ones_row[:], 1.0)
    ident = consts.tile([128, 128], BF16, tag="ident", name="ident")
    nc.sync.dma_start(out=ident[:], in_=io["ident"][:, :])
    ln_eps = consts.tile([1, 1], F32, tag="ln_eps", name="ln_eps")
    nc.vector.memset(ln_eps[:], 1e-5)

    def col_const(nm, w, key):
        t = consts.tile([128, w], F32, tag=nm, name=nm)
        nc.sync.dma_start(out=t[:], in_=io[key].rearrange("(t p) -> p t", p=128))
        return t

    inv_sc_col = col_const("inv_sc_col", NCT, "inv_sc")
    eps_sc_col = col_const("eps_sc_col", NCT, "eps_sc")
    bk_col = col_const("bk_col", NCT, "bk")
    bv_col = col_const("bv_col", NCT, "bv")
    fc2b_col = col_const("fc2b_col", NCT, "fc2_b")
    fc1b_col = col_const("fc1b_col", NCH, "fc1_b")
    dwb_col = col_const("dwb_col", NCH, "dw_b")
    ln1g_col = col_const("ln1g_col", NCH, "ln1_g")
    ln1b_col = col_const("ln1b_col", NCH, "ln1_b")
    dw_col = consts.tile([128, 9 * NCH], F32, tag="dw_col", name="dw_col")
    nc.sync.dma_start(out=dw_col[:],
                      in_=io["dw_w9"].rearrange("(t ch p) -> p (t ch)", p=128, ch=NCH))
    inv_sc_b = consts.tile([128, C], F32, tag="inv_sc_b", name="inv_sc_b")
    src = io["inv_sc"]
    nc.sync.dma_start(out=inv_sc_b[:],
                      in_=bass.AP(tensor=src.tensor, offset=src.offset,
                                  ap=[[0, 128]] + src.ap))
    if bq_nonzero:
        bq_b = consts.tile([128, C], F32, tag="bq_b", name="bq_b")
        srcq = io["bq"]
        nc.sync.dma_start(out=bq_b[:],
                          in_=bass.AP(tensor=srcq.tensor, offset=srcq.offset,
                                      ap=[[0, 128]] + srcq.ap))

    # persistent across phases
    big = ctx.enter_context(tc.tile_pool(name="big", bufs=1))
    enh = [big.tile([128, N], BF16, tag=f"enh{ct}", name=f"enh{ct}")
           for ct in range(NCT)]
    ksum_col = consts.tile([128, NCT], F32, tag="ksum_col", name="ksum_col")
    s1_col = consts.tile([128, NCT], F32, tag="s1_col", name="s1_col")

    # ================= ATTENTION SCOPE =================
    with ExitStack() as attn_ctx:
        xpool = attn_ctx.enter_context(tc.tile_pool(name="xpool", bufs=1))
        wpool = attn_ctx.enter_context(tc.tile_pool(name="wpool", bufs=1))
        epool = attn_ctx.enter_context(tc.tile_pool(name="epool", bufs=1))
        xs, wq, wk, wv = [], [], [], []
        for ct in range(NCT):
            t = xpool.tile([128, N], BF16, tag=f"x{ct}", name=f"x{ct}")
            nc.sync.dma_start(out=t[:], in_=io["x"][128 * ct:128 * (ct + 1), :])
            xs.append(t)
        for nm, lst in (("wqt", wq), ("wkt", wk), ("wvt", wv)):
            for ct in range(NCT):
                t = wpool.tile([128, C], BF16, tag=f"{nm}{ct}", name=f"{nm}{ct}")
                nc.sync.dma_start(out=t[:], in_=io[nm][128 * ct:128 * (ct + 1), :])
                lst.append(t)
        E = [epool.tile([128, N], BF16, tag=f"E{ct}", name=f"E{ct}")
             for ct in range(NCT)]
        kv_sb = [epool.tile([128, C], BF16, tag=f"kv{ct}", name=f"kv{ct}")
                 for ct in range(NCT)]

        # ---- Phase 1: K/V -> kv, ksum ----
        with tc.tile_pool(name="p1psum", bufs=1, space="PSUM") as p1psum, \
             tc.tile_pool(name="qkv_ps", bufs=2, space="PSUM") as qkv_ps, \
             tc.tile_pool(name="kwork", bufs=2) as kwork:
            kv_ps = p1psum.tile([128, 2048], F32, tag="kv_ps", name="kv_ps")
            ksum_ps = p1psum.tile([128, NCT], F32, tag="ksum_ps", name="ksum_ps")
            for bp in range(8):
                for asub in range(4):
                    it = bp * 4 + asub
                    kps = qkv_ps.tile([128, 512], F32, tag="kvps", name="kvps")
                    for ct in range(NCT):
                        nc.tensor.matmul(kps[:],
                                         wk[ct][:, 128 * asub:128 * (asub + 1)],
                                         xs[ct][:, 512 * bp:512 * (bp + 1)],
                                         start=(ct == 0), stop=(ct == NCT - 1))
                    Ek = kwork.tile([128, 512], F32, tag="Ek", name="Ek")
                    S = small.tile([128, 1], F32, tag="kS", name="kS")
                    nc.scalar.activation(Ek[:], kps[:], AF.Exp,
                                         bias=bk_col[:, asub:asub + 1], scale=1.0)
                    nc.vector.reduce_sum(S[:], Ek[:], axis=AX.X)
                    vps = qkv_ps.tile([128, 512], F32, tag="kvps", name="kvps")
                    for ct in range(NCT):
                        nc.tensor.matmul(vps[:],
                                         wv[ct][:, 128 * asub:128 * (asub + 1)],
                                         xs[ct][:, 512 * bp:512 * (bp + 1)],
                                         start=(ct == 0), stop=(ct == NCT - 1))
                    vt = kwork.tile([128, 512], BF16, tag="vt", name="vt")
                    nc.vector.tensor_scalar(vt[:], vps[:], bv_col[:, asub:asub + 1],
                                            None, op0=OP.add)
                    rS = small.tile([128, 1], F32, tag="krS", name="krS")
                    nc.vector.reciprocal(rS[:], S[:])
                    nc.vector.tensor_scalar(Ek[:], Ek[:], rS[:], 1e-6,
                                            op0=OP.mult, op1=OP.add)
                    k2 = kwork.tile([128, 512], BF16, tag="k2", name="k2")
                    nc.vector.tensor_tensor(k2[:], Ek[:], inv_sc_b[:], op=OP.mult)
                    sq = kwork.tile([128, 512], BF16, tag="ksq", name="ksq")
                    s2 = small.tile([128, 1], F32, tag="ks2", name="ks2")
                    nc.scalar.activation(sq[:], k2[:], AF.Square)
                    nc.vector.reduce_sum(s2[:], sq[:], axis=AX.X)
                    k3 = kwork.tile([128, 512], BF16, tag="k3", name="k3")
                    nc.vector.tensor_tensor(k3[:], sq[:], k2[:], op=OP.mult)
                    sq6 = kwork.tile([128, 512], BF16, tag="ksq", name="ksq")
                    s6 = small.tile([128, 1], F32, tag="ks6", name="ks6")
                    nc.scalar.activation(sq6[:], k3[:], AF.Square)
                    nc.vector.reduce_sum(s6[:], sq6[:], axis=AX.X)
                    rs2 = small.tile([128, 1], F32, tag="krs2", name="krs2")
                    nc.scalar.activation(rs2[:], s2[:], AF.Sqrt)
                    rs6 = small.tile([128, 1], F32, tag="krs6", name="krs6")
                    nc.scalar.activation(rs6[:], s6[:], AF.Sqrt)
                    rr6 = small.tile([128, 1], F32, tag="krr6", name="krr6")
                    nc.vector.reciprocal(rr6[:], rs6[:])
                    fs = small.tile([128, 1], F32, tag="kfs", name="kfs")
                    nc.vector.tensor_tensor(fs[:], rs2[:], rr6[:], op=OP.mult)
                    kf = kwork.tile([128, 512], BF16, tag="kfocus", name="kfocus")
                    nc.vector.tensor_scalar(kf[:], k3[:], fs[:], None, op0=OP.mult)
                    for cs in range(4):
                        nc.tensor.matmul(kv_ps[:, 512 * cs:512 * (cs + 1)],
                                         kf[:, 128 * cs:128 * (cs + 1)], vt[:],
                                         start=(it == 0), stop=(it == 31),
                                         skip_group_check=True)
                        nc.tensor.matmul(ksum_ps[:, cs:cs + 1],
                                         kf[:, 128 * cs:128 * (cs + 1)], ones_col[:],
                                         start=(it == 0 and cs == 0),
                                         stop=(it == 31 and cs == 3),
                                         skip_group_check=True)
            for ct in range(NCT):
                nc.scalar.activation(kv_sb[ct][:], kv_ps[:, 512 * ct:512 * (ct + 1)],
                                     AF.Copy)
            nc.scalar.activation(ksum_col[:], ksum_ps[:], AF.Copy)

            # ---- Phase 2A: Q_pre -> E (pi-order), S ----
            Spart = [small.tile([128, 8], F32, tag=f"Spart{ct}", name=f"Spart{ct}")
                     for ct in range(NCT)]
            for bp in range(8):
                for cs in range(NCT):
                    qps = qkv_ps.tile([128, 512], F32, tag="kvps", name="kvps")
                    for ct in range(NCT):
                        nc.tensor.matmul(
                            qps[:],
                            xs[ct][:, 512 * bp + 128 * cs:512 * bp + 128 * (cs + 1)],
                            wq[ct][:],
                            start=(ct == 0), stop=(ct == NCT - 1))
                    if bq_nonzero:
                        nc.vector.tensor_tensor(qps[:], qps[:], bq_b[:], op=OP.add)
                    nc.scalar.activation(E[cs][:, 512 * bp:512 * (bp + 1)], qps[:],
                                         AF.Exp)
                    nc.vector.reduce_sum(Spart[cs][:, bp:bp + 1],
                                         E[cs][:, 512 * bp:512 * (bp + 1)],
                                         axis=AX.X)
        for ct in range(NCT):
            Ssum = small.tile([128, 1], F32, tag="qSsum", name="qSsum")
            nc.vector.reduce_sum(Ssum[:], Spart[ct][:], axis=AX.X)
            rS = small.tile([128, 1], F32, tag="qrS", name="qrS")
            nc.vector.reciprocal(rS[:], Ssum[:])
            nc.vector.tensor_tensor(s1_col[:, ct:ct + 1], rS[:],
                                    inv_sc_col[:, ct:ct + 1], op=OP.mult)

        # ---- Phase 2B: q chain + attn per pi-chunk ----
        with tc.tile_pool(name="qstat_ps", bufs=1, space="PSUM") as qstat_ps, \
             tc.tile_pool(name="attn_ps", bufs=2, space="PSUM") as attn_ps, \
             tc.tile_pool(name="qwork", bufs=2) as qwork:
            for bp in range(8):
                Aps = qstat_ps.tile([1, 512], F32, tag="Aps", name="Aps")
                Bps = qstat_ps.tile([1, 512], F32, tag="Bps", name="Bps")
                Cps = qstat_ps.tile([1, 512], F32, tag="Cps", name="Cps")
                sl = slice(512 * bp, 512 * (bp + 1))
                for ct in range(NCT):
                    q2 = qwork.tile([128, 512], BF16, tag="q2", name="q2")
                    nc.vector.tensor_scalar(q2[:], E[ct][:, sl],
                                            s1_col[:, ct:ct + 1],
                                            eps_sc_col[:, ct:ct + 1],
                                            op0=OP.mult, op1=OP.add)
                    sq = qwork.tile([128, 512], BF16, tag="qsq", name="qsq")
                    nc.scalar.activation(sq[:], q2[:], AF.Square)
                    nc.tensor.matmul(Aps[:], ones_col[:], sq[:],
                                     start=(ct == 0), stop=(ct == NCT - 1),
                                     skip_group_check=True)
                    nc.vector.tensor_tensor(E[ct][:, sl], sq[:], q2[:], op=OP.mult)
                    q6 = qwork.tile([128, 512], BF16, tag="q6", name="q6")
                    nc.scalar.activation(q6[:], E[ct][:, sl], AF.Square)
                    nc.tensor.matmul(Bps[:], ones_col[:], q6[:],
                                     start=(ct == 0), stop=(ct == NCT - 1),
                                     skip_group_check=True)
                    qk = qwork.tile([128, 512], BF16, tag="qk", name="qk")
                    nc.vector.tensor_scalar(qk[:], E[ct][:, sl],
                                            ksum_col[:, ct:ct + 1], None,
                                            op0=OP.mult)
                    nc.tensor.matmul(Cps[:], ones_col[:], qk[:],
                                     start=(ct == 0), stop=(ct == NCT - 1),
                                     skip_group_check=True)
                rsA = rowp.tile([1, 512], F32, tag="rsA", name="rsA")
                nc.scalar.activation(rsA[:], Aps[:], AF.Sqrt)
                rsB = rowp.tile([1, 512], F32, tag="rsB", name="rsB")
                nc.scalar.activation(rsB[:], Bps[:], AF.Sqrt)
                rB = rowp.tile([1, 512], F32, tag="rB", name="rB")
                nc.vector.reciprocal(rB[:], rsB[:])
                rowscale = rowp.tile([1, 512], F32, tag="rowscale", name="rowscale")
                nc.vector.tensor_tensor(rowscale[:], rsA[:], rB[:], op=OP.mult)
                den = rowp.tile([1, 512], F32, tag="den", name="den")
                nc.vector.scalar_tensor_tensor(den[:], Cps[:], 1.0, rowscale[:],
                                               op0=OP.mult, op1=OP.mult)
                den2 = rowp.tile([1, 512], F32, tag="den2", name="den2")
                nc.vector.tensor_scalar(den2[:], den[:], 1e-6, None, op0=OP.add)
                rden = rowp.tile([1, 512], F32, tag="rden", name="rden")
                nc.vector.reciprocal(rden[:], den2[:])
                total = rowp.tile([1, 512], BF16, tag="total", name="total")
                nc.vector.tensor_tensor(total[:], rowscale[:], rden[:], op=OP.mult)
                total_b = qstat_ps.tile([128, 512], F32, tag="bcq", name="bcq")
                _bcast(nc, ones_row, total_b[:], total[:])
                for ct in range(NCT):
                    nc.vector.tensor_tensor(E[ct][:, sl], E[ct][:, sl], total_b[:],
                                            op=OP.mult)
                for ds in range(NCT):
                    aps = attn_ps.tile([128, 512], F32, tag="aps", name="aps")
                    for ct in range(NCT):
                        nc.tensor.matmul(aps[:],
                                         kv_sb[ct][:, 128 * ds:128 * (ds + 1)],
                                         E[ct][:, sl],
                                         start=(ct == 0), stop=(ct == NCT - 1))
                    enh_v = enh[ds][:].rearrange("p (a b) -> p b a", b=8)[:, bp, :]
                    x_v = xs[ds][:].rearrange("p (a b) -> p b a", b=8)[:, bp, :]
                    nc.vector.tensor_tensor(enh_v, aps[:], x_v, op=OP.add)

    # ================= FFN SCOPE =================
    fcw = ctx.enter_context(tc.tile_pool(name="fcw", bufs=1))
    fc1w = [fcw.tile([128, HID], BF16, tag=f"fc1w{ct}", name=f"fc1w{ct}")
            for ct in range(NCT)]
    for ct in range(NCT):
        nc.sync.dma_start(out=fc1w[ct][:], in_=io["fc1_w"][128 * ct:128 * (ct + 1), :])
    fc2w = [fcw.tile([128, C], BF16, tag=f"fc2w{ch}", name=f"fc2w{ch}")
            for ch in range(NCH)]
    for ch in range(NCH):
        nc.sync.dma_start(out=fc2w[ch][:], in_=io["fc2_w"][128 * ch:128 * (ch + 1), :])
    tqpool = ctx.enter_context(tc.tile_pool(name="tqpool", bufs=1))
    apad_pool = ctx.enter_context(tc.tile_pool(name="apad", bufs=1))
    a_pad = [apad_pool.tile([128, 18 * PW], BF16, tag=f"apad{ch}", name=f"apad{ch}")
             for ch in range(NCH)]
    for ch in range(NCH):
        nc.vector.memset(a_pad[ch][:], 0.0)
    upool = ctx.enter_context(tc.tile_pool(name="upool", bufs=1))
    u_t = [upool.tile([128, 1024], BF16, tag=f"u{ch}", name=f"u{ch}")
           for ch in range(NCH)]

    out_ap = io["out"]
    with tc.tile_pool(name="a_ps", bufs=1, space="PSUM") as a_ps, \
         tc.tile_pool(name="l1_ps", bufs=1, space="PSUM") as l1_ps, \
         tc.tile_pool(name="fc2_ps", bufs=1, space="PSUM") as fc2_ps, \
         tc.tile_pool(name="tr_ps", bufs=1, space="PSUM") as tr_ps, \
         tc.tile_pool(name="fwork", bufs=2) as fwork, \
         tc.tile_pool(name="outp", bufs=2) as outp:
        for q in range(4):
            h0 = max(0, 16 * q - 1)
            h1 = min(64, 16 * q + 17)
            if q == 3:
                for ch in range(NCH):
                    nc.vector.memset(
                        a_pad[ch][:].rearrange("p (r w) -> p r w", w=PW)[:, 17, :],
                        0.0)
            n0, n1 = 64 * h0, 64 * h1
            pieces = []
            pos = n0
            while pos < n1:
                ln = min(512, n1 - pos)
                pieces.append((pos, ln))
                pos += ln
            # ---- t_q = LN_mlp(enh) for this quarter's n-range ----
            t_q = [tqpool.tile([128, 1152], BF16, tag=f"tq{ct}", name=f"tq{ct}")
                   for ct in range(NCT)]
            for (pn, ln) in pieces:
                Sm = l1_ps.tile([1, 512], F32, tag="S1", name="S1")
                Sm2 = l1_ps.tile([1, 512], F32, tag="S12", name="S12")
                for ct in range(NCT):
                    esl = enh[ct][:, pn:pn + ln]
                    sq = fwork.tile([128, 512], BF16, tag="usq", name="usq")
                    nc.scalar.activation(sq[:, :ln], esl, AF.Square)
                    nc.tensor.matmul(Sm[:, :ln], ones_col[:], esl,
                                     start=(ct == 0), stop=(ct == NCT - 1),
                                     skip_group_check=True)
                    nc.tensor.matmul(Sm2[:, :ln], ones_col[:], sq[:, :ln],
                                     start=(ct == 0), stop=(ct == NCT - 1),
                                     skip_group_check=True)
                r_rowb, m_rowb = _row_norm_stats(nc, rowp, Sm[:, :ln],
                                                 Sm2[:, :ln], 1.0 / C, ln_eps, ln)
                r_b = l1_ps.tile([128, 512], F32, tag="bcf", name="bcf")
                _bcast(nc, ones_row, r_b[:, :ln], r_rowb[:, :ln])
                m_b = l1_ps.tile([128, 512], F32, tag="bcf2", name="bcf2")
                _bcast(nc, ones_row, m_b[:, :ln], m_rowb[:, :ln])
                for ct in range(NCT):
                    p = fwork.tile([128, 512], BF16, tag="fp", name="fp")
                    nc.vector.tensor_tensor(p[:, :ln], enh[ct][:, pn:pn + ln],
                                            m_b[:, :ln], op=OP.subtract)
                    nc.vector.tensor_tensor(t_q[ct][:, pn - n0:pn - n0 + ln],
                                            p[:, :ln], r_b[:, :ln], op=OP.mult)
            if dbg and q == 0:
                for ct in range(NCT):
                    nc.gpsimd.dma_start(out=io["dbg_t"][128 * ct:128 * (ct + 1), :1088],
                                        in_=t_q[ct][:, :1088])
            for ch in range(NCH):
                apr = a_pad[ch][:].rearrange("p (r w) -> p r w", w=PW)
                for (pn, ln) in pieces:
                    aps_t = a_ps.tile([128, 512], F32, tag="apsq", name="apsq")
                    for ct in range(NCT):
                        nc.tensor.matmul(aps_t[:, :ln],
                                         fc1w[ct][:, 128 * ch:128 * (ch + 1)],
                                         t_q[ct][:, pn - n0:pn - n0 + ln],
                                         start=(ct == 0), stop=(ct == NCT - 1))
                    rr0 = pn // 64 - (16 * q - 1)
                    nrow = ln // 64
                    nc.vector.tensor_scalar(
                        apr[:, rr0:rr0 + nrow, 1:65],
                        aps_t[:, :ln].rearrange("p (r w) -> p r w", w=64),
                        fc1b_col[:, ch:ch + 1], None, op0=OP.add)
                acc = fwork.tile([128, 1024], BF16, tag="dwacc", name="dwacc", bufs=1)
                acc2 = fwork.tile([128, 1024], BF16, tag="dwacc2", name="dwacc2", bufs=1)
                uv = u_t[ch][:].rearrange("p (r w) -> p r w", w=64)
                first = True
                cur, nxt = acc, acc2
                for dy in range(3):
                    for dx in range(3):
                        ti = (3 * dy + dx) * NCH + ch
                        tap = dw_col[:, ti:ti + 1]
                        inv = apr[:, dy:dy + 16, dx:dx + 64]
                        if first:
                            nc.vector.tensor_scalar(
                                acc[:].rearrange("p (r w) -> p r w", w=64),
                                inv, tap, None, op0=OP.mult)
                            first = False
                        else:
                            nc.vector.scalar_tensor_tensor(
                                nxt[:].rearrange("p (r w) -> p r w", w=64),
                                inv, tap,
                                cur[:].rearrange("p (r w) -> p r w", w=64),
                                op0=OP.mult, op1=OP.add)
                            cur, nxt = nxt, cur
                nc.vector.scalar_tensor_tensor(
                    uv, cur[:].rearrange("p (r w) -> p r w", w=64),
                    dwb_col[:, ch:ch + 1], apr[:, 1:17, 1:65],
                    op0=OP.add, op1=OP.add)
                if dbg and q == 0 and ch == 0:
                    nc.gpsimd.dma_start(out=io["dbg_a"][:, :], in_=a_pad[0][:, :])
                    nc.gpsimd.dma_start(out=io["dbg_u"][:, :], in_=u_t[0][:, :])
            for cj in range(2):
                S1 = l1_ps.tile([1, 512], F32, tag="S1", name="S1")
                S12 = l1_ps.tile([1, 512], F32, tag="S12", name="S12")
                for ch in range(NCH):
                    usl = u_t[ch][:, 512 * cj:512 * (cj + 1)]
                    sq = fwork.tile([128, 512], BF16, tag="usq", name="usq")
                    nc.vector.tensor_tensor(sq[:], usl, usl, op=OP.mult)
                    nc.tensor.matmul(S1[:], ones_col[:], usl,
                                     start=(ch == 0), stop=(ch == NCH - 1),
                                     skip_group_check=True)
                    nc.tensor.matmul(S12[:], ones_col[:], sq[:],
                                     start=(ch == 0), stop=(ch == NCH - 1),
                                     skip_group_check=True)
                r_rowb, m_rowb = _row_norm_stats(nc, rowp, S1[:], S12[:],
                                                 1.0 / HID, ln_eps, 512)
                r_b = l1_ps.tile([128, 512], F32, tag="bcf", name="bcf")
                _bcast(nc, ones_row, r_b[:], r_rowb[:])
                m_b = l1_ps.tile([128, 512], F32, tag="bcf2", name="bcf2")
                _bcast(nc, ones_row, m_b[:], m_rowb[:])
                for ch in range(NCH):
                    usl = u_t[ch][:, 512 * cj:512 * (cj + 1)]
                    p = fwork.tile([128, 512], BF16, tag="fp", name="fp")
                    nc.vector.tensor_tensor(p[:], usl, m_b[:], op=OP.subtract)
                    vv = fwork.tile([128, 512], BF16, tag="fvv", name="fvv")
                    nc.vector.tensor_tensor(vv[:], p[:], r_b[:], op=OP.mult)
                    nc.scalar.activation(usl, vv[:], AF.Gelu,
                                         bias=ln1b_col[:, ch:ch + 1],
                                         scale=ln1g_col[:, ch:ch + 1])
                nq = 1024 * q + 512 * cj
                finals = []
                for csp in range(2):
                    f2 = fc2_ps.tile([128, 1024], F32, tag="f2", name="f2")
                    for ci in range(2):
                        cs = 2 * csp + ci
                        for ch in range(NCH):
                            nc.tensor.matmul(f2[:, 512 * ci:512 * (ci + 1)],
                                             fc2w[ch][:, 128 * cs:128 * (cs + 1)],
                                             u_t[ch][:, 512 * cj:512 * (cj + 1)],
                                             start=(ch == 0), stop=(ch == NCH - 1),
                                             skip_group_check=True)
                    for ci in range(2):
                        cs = 2 * csp + ci
                        fin = fwork.tile([128, 512], BF16, tag=f"fin{cs}",
                                         name=f"fin{cs}", bufs=1)
                        nc.vector.scalar_tensor_tensor(
                            fin[:], f2[:, 512 * ci:512 * (ci + 1)],
                            fc2b_col[:, cs:cs + 1], enh[cs][:, nq:nq + 512],
                            op0=OP.add, op1=OP.add)
                        finals.append(fin)
                for ns in range(4):
                    trp = tr_ps.tile([128, 512], BF16, tag="trp", name="trp")
                    for cs in range(NCT):
                        nc.tensor.transpose(trp[:, 128 * cs:128 * (cs + 1)],
                                            finals[cs][:, 128 * ns:128 * (ns + 1)],
                                            ident[:])
                    ot = outp.tile([128, 512], F32, tag="ot", name="ot")
                    nc.scalar.activation(ot[:], trp[:], AF.Copy)
                    nc.sync.dma_start(
                        out=out_ap[nq + 128 * ns:nq + 128 * (ns + 1), :],
                        in_=ot[:])


# ---------------------------------------------------------------------------
# host wrapper
# ---------------------------------------------------------------------------
_CACHE = {}


def _build(bq_nonzero, dbg=False):
    key = (bq_nonzero, dbg)
    if key in _CACHE:
        return _CACHE[key]
    nc = bacc.Bacc("TRN2", target_bir_lowering=False, debug=False)
    io = {}
    io["x"] = nc.dram_tensor("x", [C, N], BF16, kind="ExternalInput").ap()
    for nm in ("wqt", "wkt", "wvt"):
        io[nm] = nc.dram_tensor(nm, [C, C], BF16, kind="ExternalInput").ap()
    io["ident"] = nc.dram_tensor("ident", [128, 128], BF16, kind="ExternalInput").ap()
    for nm in ("inv_sc", "eps_sc", "bk", "bv", "fc2_b"):
        io[nm] = nc.dram_tensor(nm, [C], F32, kind="ExternalInput").ap()
    for nm in ("fc1_b", "dw_b", "ln1_g", "ln1_b"):
        io[nm] = nc.dram_tensor(nm, [HID], F32, kind="ExternalInput").ap()
    io["dw_w9"] = nc.dram_tensor("dw_w9", [9 * HID], F32, kind="ExternalInput").ap()
    io["fc1_w"] = nc.dram_tensor("fc1_w", [C, HID], BF16, kind="ExternalInput").ap()
    io["fc2_w"] = nc.dram_tensor("fc2_w", [HID, C], BF16, kind="ExternalInput").ap()
    if bq_nonzero:
        io["bq"] = nc.dram_tensor("bq", [C], F32, kind="ExternalInput").ap()
    io["out"] = nc.dram_tensor("out", [N, C], F32, kind="ExternalOutput").ap()
    if dbg:
        io["dbg_t"] = nc.dram_tensor("dbg_t", [C, 1152], F32,
                                     kind="ExternalOutput").ap()
        io["dbg_a"] = nc.dram_tensor("dbg_a", [128, 18 * PW], F32,
                                     kind="ExternalOutput").ap()
        io["dbg_u"] = nc.dram_tensor("dbg_u", [128, 1024], F32,
                                     kind="ExternalOutput").ap()
    with tile.TileContext(nc) as tc:
        emit(tc, io, bq_nonzero, dbg)
    nc.compile()
    _CACHE[key] = (nc, io)
    return nc, io


def kernel(x, Wq, bq, Wk, bk, Wv, bv, scale, fc1_w, fc1_b, dw_w, dw_b,
           fc2_w, fc2_b, ln1_g, ln1_b, lnm_g, lnm_b):
    from concourse.bass_utils import run_bass_kernel_spmd

    x = np.asarray(x, np.float32)
    f = lambda a: np.asarray(a, np.float32)
    Wq, Wk, Wv = f(Wq), f(Wk), f(Wv)
    bq, bk, bv = f(bq).ravel(), f(bk).ravel(), f(bv).ravel()
    fc1_w, fc1_b = f(fc1_w), f(fc1_b)
    dw_w, dw_b = f(dw_w), f(dw_b)
    fc2_w, fc2_b = f(fc2_w), f(fc2_b)
    ln1_g, ln1_b = f(ln1_g), f(ln1_b)
    lnm_g, lnm_b = f(lnm_g), f(lnm_b)
    sc = np.log1p(np.exp(f(scale))).ravel()
    inv_sc = (1.0 / sc).astype(np.float32)
    eps_sc = (1e-6 * inv_sc).astype(np.float32)
    fc1w_eff = (lnm_g[:, None] * fc1_w).astype(BFNP)
    fc1b_eff = (fc1_b + lnm_b @ fc1_w).astype(np.float32)
    dw9 = np.ascontiguousarray(dw_w.reshape(HID, 9).T).ravel().astype(np.float32)
    bq_nonzero = bool(np.any(bq != 0.0))

    nc, _ = _build(bq_nonzero)
    shared = {
        "wqt": np.ascontiguousarray(Wq.T).astype(BFNP),
        "wkt": np.ascontiguousarray(Wk.T).astype(BFNP),
        "wvt": np.ascontiguousarray(Wv.T).astype(BFNP),
        "ident": np.eye(128, dtype=BFNP),
        "inv_sc": inv_sc, "eps_sc": eps_sc, "bk": bk, "bv": bv,
        "fc2_b": fc2_b, "fc1_b": fc1b_eff, "dw_b": dw_b,
        "ln1_g": ln1_g, "ln1_b": ln1_b, "dw_w9": dw9,
        "fc1_w": fc1w_eff, "fc2_w": fc2_w.astype(BFNP),
    }
    if bq_nonzero:
        shared["bq"] = bq
    in_maps = []
    for i in range(B):
        m = dict(shared)
        m["x"] = np.ascontiguousarray(x[i].reshape(C, N)).astype(BFNP)
        in_maps.append(m)
    res = run_bass_kernel_spmd(nc, in_maps, core_ids=list(range(B)))
    out = np.stack([r["out"] for r in res.results], axis=0)
    return out.reshape(B, H, W, C).astype(np.float32)
